# revision 1
# baseline (speedup 1.0000x reference)
"""Trainium2 Bass kernel for nn_BareDotProdAttnEncoder (tree scan, gnn_message_passing).

Reference semantics (per batch element b):
  h_0 = x_0
  for i in 1..N-1:
      p = parent[i]  (p < i)
      alpha = exp(<h_p, x_i>); beta = exp(<x_i, x_i>)
      h_i = (alpha*h_p + beta*x_i) / (alpha + beta + 1e-15)

Equivalent form used on device:
  w = sigmoid(<h_p, x_i> - <x_i, x_i>)      (= alpha/(alpha+beta))
  h_i = w*(h_p - x_i) + x_i

Strategy: the recurrence only couples a node to its parent, and with
parent[i] < i drawn uniformly the trees are shallow (~18 levels for
N=2048). Process nodes level-by-level: all nodes of one level are
independent given the previous levels' h. All indices are known on the
host, so the host computes a level schedule and the device does batched
index gathers (SWDGE dma_gather), the dot/sigmoid/blend math, and
contiguous writebacks of each level into a persistent HBM state buffer
laid out level-contiguously ("sorted" order). The host inverse-permutes
the returned state into the reference node order.

Sharding: pure data parallelism over the batch; each of the 8 cores owns
4 trees, processed as 2 independent streams of 2 trees each (streams
pipeline against each other to hide per-level DMA latency).
"""

import os
import numpy as np

N_CORES = 8
STREAMS = int(os.environ.get("K_STREAMS", "2"))
TREES_PER_STREAM = 4 // STREAMS
DIM = 512
PART = 128
XBUFS = int(os.environ.get("K_XBUFS", "2"))
PBUFS = int(os.environ.get("K_PBUFS", "1"))
DBUFS = int(os.environ.get("K_DBUFS", "1"))
HBUFS = int(os.environ.get("K_HBUFS", "2"))
DSUB_ENG = os.environ.get("K_DSUB_ENG", "vector")
ABLATE = os.environ.get("K_ABLATE", "")
MAXLEV = int(os.environ.get("K_MAXLEV", "0"))  # 0 = all levels
SKIP = set(x for x in os.environ.get("K_SKIP", "").split(",") if x)
REPEAT = int(os.environ.get("K_REPEAT", "1"))
DYN = os.environ.get("K_DYN", "1") == "1"  # dynamic gather counts (skip pad traffic)
SINGLE_PACKET = os.environ.get("K_SINGLEPKT", "1") == "1"
WBSCATTER = os.environ.get("K_WBSCATTER", "0") == "1"  # exact-row wb via scatter-add


def _compute_depths(conn):
    B, N = conn.shape
    depths = np.zeros((B, N), np.int32)
    bidx = np.arange(B)
    for i in range(1, N):
        depths[:, i] = depths[bidx, conn[:, i]] + 1
    return depths


def _assign_trees(S, B):
    """Group trees into (stream, core) slots to minimize total padded chunks.
    S: per-tree level-size matrix [B, L]. Returns groups[g][c] = tuple of trees.
    Deterministic local search (seeded)."""
    L = S.shape[1]
    tps = TREES_PER_STREAM
    nslots = B // tps  # STREAMS * N_CORES
    nat = [tuple(range(tps * s, tps * (s + 1))) for s in range(nslots)]

    def cost(assign):
        tot = 0
        for g in range(STREAMS):
            lv = np.zeros(L, np.int64)
            for c in range(N_CORES):
                grp = assign[g * N_CORES + c]
                n = np.sum(S[list(grp)], axis=0)
                lv = np.maximum(lv, (n + PART - 1) // PART)
            tot += lv.sum()
        return int(tot)

    if os.environ.get("K_NATASSIGN", "0") == "1":
        return [[nat[g * N_CORES + c] for c in range(N_CORES)] for g in range(STREAMS)]
    rng = np.random.default_rng(12345)
    cur = [list(p) for p in nat]
    cc = cost([tuple(p) for p in cur])
    best, bc = [tuple(p) for p in cur], cc
    for _ in range(20000):
        a = int(rng.integers(0, nslots)); b2 = int(rng.integers(0, nslots))
        if a == b2:
            continue
        i = int(rng.integers(0, tps)); j = int(rng.integers(0, tps))
        cur[a][i], cur[b2][j] = cur[b2][j], cur[a][i]
        c2 = cost([tuple(p) for p in cur])
        if c2 <= cc:
            cc = c2
            if c2 < bc:
                best, bc = [tuple(p) for p in cur], c2
        else:
            cur[a][i], cur[b2][j] = cur[b2][j], cur[a][i]
    return [[best[g * N_CORES + c] for c in range(N_CORES)] for g in range(STREAMS)]


def _build_schedule(conn):
    """Host-side schedule: level structure, per-core index arrays, maps.

    Returns (L, Cls, sched) where
      L: number of levels
      Cls[g]: list of per-level chunk counts (uniform across cores)
      sched[c]: dict with per-core input arrays + posmat for assembly
    """
    B, N = conn.shape
    depths = _compute_depths(conn)
    L = int(depths.max()) + 1

    # node lists per (batch, level), ordered by node id (stable)
    order = [[np.nonzero(depths[b] == l)[0] for l in range(L)] for b in range(B)]

    S = np.zeros((B, L), np.int64)
    for b in range(B):
        S[b] = np.bincount(depths[b], minlength=L)
    groups = _assign_trees(S, B)  # groups[g][c] = tree tuple

    # uniform chunk capacities per stream
    Cls = []
    for g in range(STREAMS):
        Cl = np.zeros(L, np.int64)
        for c in range(N_CORES):
            trees = groups[g][c]
            for l in range(L):
                n = sum(len(order[b][l]) for b in trees)
                Cl[l] = max(Cl[l], (n + PART - 1) // PART)
        Cls.append([int(x) for x in Cl])

    sched = []
    for c in range(N_CORES):
        entry = {}
        for g in range(STREAMS):
            Cl = Cls[g]
            sumC = sum(Cl)
            R = PART * sumC
            trees = groups[g][c]
            pad = np.int16(-1 if DYN else 0)
            eidx = np.full(R, pad, np.int16)   # row -> embedding row (t*N + i)
            pidx = np.full(R, pad, np.int16)   # row -> parent state row
            cnt = np.zeros(L, np.int32)        # real rows per level (min 1)
            posmat = np.zeros((TREES_PER_STREAM, N), np.int32)  # node -> state row
            off = 0
            for l in range(L):
                base = PART * off
                j = 0
                for t, b in enumerate(trees):
                    for i in order[b][l]:
                        row = base + j
                        eidx[row] = t * N + i
                        posmat[t, i] = row
                        if l > 0:
                            pidx[row] = posmat[t, conn[b, i]]
                        j += 1
                assert j <= PART * Cl[l]
                if j == 0 and Cl[l] > 0:
                    eidx[base] = 0
                    pidx[base] = 0
                    j = 1
                cnt[l] = j
                off += Cl[l]

            def wrap(vals):
                # gather index layout: within a call of num_idxs n, index j
                # lives at [j%16, j//16]; replicate across the 8 groups of
                # 16 partitions. Calls slice per-level column blocks.
                out = np.zeros((PART, 8 * sumC), np.int16)
                o = 0
                for l in range(L):
                    n = PART * Cl[l]
                    block = vals[PART * o : PART * o + n].reshape(8 * Cl[l], 16).T  # [16, 8C]
                    for rep in range(8):
                        out[16 * rep : 16 * (rep + 1), 8 * o : 8 * (o + Cl[l])] = block
                    o += Cl[l]
                return out

            widx = np.full(R, pad, np.int16)   # row -> its own state row (for scatter wb)
            o2 = 0
            for l in range(L):
                nvalid = cnt[l]
                base = PART * o2
                widx[base : base + nvalid] = np.arange(base, base + nvalid, dtype=np.int16)
                o2 += Cl[l]
            entry[f"eidx{g}"] = wrap(eidx)
            entry[f"pidx{g}"] = wrap(pidx)
            entry[f"widx{g}"] = wrap(widx)
            entry[f"cnt{g}"] = cnt.reshape(1, L)
            entry[f"posmat{g}"] = posmat
            entry[f"trees{g}"] = list(trees)
        sched.append(entry)
    return L, Cls, sched


def _build_program(L, Cls):
    import concourse.bacc as bacc
    import concourse.mybir as mybir
    import concourse.tile as tile

    f32 = mybir.dt.float32
    i16 = mybir.dt.int16
    i32 = mybir.dt.int32
    Alu = mybir.AluOpType
    Act = mybir.ActivationFunctionType

    nc = bacc.Bacc("TRN2", debug=False)

    emb_t, eidx_t, pidx_t, cnt_t, state_t, widx_t = [], [], [], [], [], []
    for g in range(STREAMS):
        sumC = sum(Cls[g])
        R = PART * sumC
        emb_t.append(nc.dram_tensor(f"emb{g}", [TREES_PER_STREAM * 2048, DIM], f32,
                                    kind="ExternalInput"))
        eidx_t.append(nc.dram_tensor(f"eidx{g}", [PART, 8 * sumC], i16,
                                     kind="ExternalInput"))
        pidx_t.append(nc.dram_tensor(f"pidx{g}", [PART, 8 * sumC], i16,
                                     kind="ExternalInput"))
        cnt_t.append(nc.dram_tensor(f"cnt{g}", [1, L], i32, kind="ExternalInput"))
        if WBSCATTER:
            widx_t.append(nc.dram_tensor(f"widx{g}", [PART, 8 * sumC], i16,
                                         kind="ExternalInput"))
        state_t.append(nc.dram_tensor(f"state{g}", [R, DIM], f32,
                                      kind="ExternalOutput"))
    fake_t = None
    if ABLATE == "nodep":
        fake_t = nc.dram_tensor("fake", [PART * max(sum(C) for C in Cls), DIM], f32)

    with tile.TileContext(nc) as tc:
        from contextlib import ExitStack
        stack = ExitStack()
        pools = []
        for g in range(STREAMS):
            p = {
                "X": stack.enter_context(tc.tile_pool(name=f"X{g}", bufs=XBUFS)),
                "P": stack.enter_context(tc.tile_pool(name=f"P{g}", bufs=PBUFS)),
                "D": stack.enter_context(tc.tile_pool(name=f"D{g}", bufs=DBUFS)),
                "H": stack.enter_context(tc.tile_pool(name=f"H{g}", bufs=HBUFS)),
                "S": stack.enter_context(tc.tile_pool(name=f"S{g}", bufs=2)),
                "I": stack.enter_context(tc.tile_pool(name=f"I{g}", bufs=1)),
            }
            pools.append(p)

        # preload index arrays, allocate junk tiles
        idxs = []
        for g in range(STREAMS):
            sumC = sum(Cls[g])
            ei = pools[g]["I"].tile([PART, 8 * sumC], i16, tag=f"ei{g}")
            pi = pools[g]["I"].tile([PART, 8 * sumC], i16, tag=f"pi{g}")
            jt = pools[g]["I"].tile([PART, DIM], f32, tag=f"jt{g}")   # dot-product junk out
            nc.sync.dma_start(ei[:, :], eidx_t[g][:, :])
            nc.sync.dma_start(pi[:, :], pidx_t[g][:, :])
            wi = None
            if WBSCATTER:
                wi = pools[g]["I"].tile([PART, 8 * sumC], i16, tag=f"wi{g}")
                nc.sync.dma_start(wi[:, :], widx_t[g][:, :])
            cr = None
            if DYN:
                ct = pools[g]["I"].tile([1, L], i32, tag=f"ct{g}")
                nc.sync.dma_start(ct[:, :], cnt_t[g][:, :])
                # one register per level: reusing one would be a WAR hazard
                # under Tile reordering (gather reads reg at exec time)
                regs = [nc.gpsimd.alloc_register(f"cnt{g}_{l}") for l in range(L)]
                cr = (ct, regs)
            idxs.append((ei, pi, jt, cr, wi))

        Luse = min(L, MAXLEV) if MAXLEV else L
        STAGGER = os.environ.get("K_STAGGER", "0") == "1"
        for _rep in range(REPEAT):
          offs = [0 for _ in range(STREAMS)]
          if STAGGER:
            # emit (g, level) waves with stream g delayed by g levels, so the
            # streams' DMA/compute phases interleave rather than collide
            waves = []
            for w in range(Luse + STREAMS - 1):
                for g in range(STREAMS):
                    l = w - g
                    if 0 <= l < Luse:
                        waves.append((l, g))
            order = waves
          else:
            order = [(l, g) for l in range(Luse) for g in range(STREAMS)]
          for l, g in order:
            if True:
                C = Cls[g][l]
                if C == 0:
                    continue
                off = offs[g]
                offs[g] += C
                ei, pi, jt, cr, wi = idxs[g]
                p = pools[g]
                n = PART * C
                if DYN:
                    ct, regs = cr
                    nc.gpsimd.reg_load(regs[l], ct[0:1, l : l + 1])
                    nreg = regs[l]
                else:
                    nreg = n

                X = p["X"].tile([PART, C, DIM], f32, tag=f"X{g}")
                H = p["H"].tile([PART, C, DIM], f32, tag=f"H{g}")

                nc.gpsimd.dma_gather(
                    X[:, :, :], emb_t[g][:, :],
                    ei[:, 8 * off : 8 * (off + C)], n, nreg, DIM,
                    single_packet=SINGLE_PACKET)

                if l == 0:
                    nc.scalar.activation(H[:, :, :], X[:, :, :], Act.Copy)
                elif ABLATE == "nocompute":
                    P = p["P"].tile([PART, C, DIM], f32, tag=f"P{g}")
                    nc.gpsimd.dma_gather(
                        P[:, :, :], state_t[g][:, :],
                        pi[:, 8 * off : 8 * (off + C)], n, n, DIM)
                    nc.scalar.activation(H[:, :, :], P[:, :, :], Act.Copy)
                else:
                    P = p["P"].tile([PART, C, DIM], f32, tag=f"P{g}")
                    D = p["D"].tile([PART, C, DIM], f32, tag=f"D{g}")
                    dp = p["S"].tile([PART, C], f32, tag=f"dp{g}")
                    wh = p["S"].tile([PART, C], f32, tag=f"wh{g}")

                    gsrc = emb_t[g] if "pgemb" in SKIP else (
                        fake_t if ABLATE == "nodep" else state_t[g])
                    nc.gpsimd.dma_gather(
                        P[:, :, :], gsrc[:, :],
                        pi[:, 8 * off : 8 * (off + C)], n, nreg, DIM,
                        single_packet=SINGLE_PACKET)

                    # D = h_p - x
                    if "tt" in SKIP:
                        nc.scalar.activation(D[:, :, :], P[:, :, :], Act.Copy)
                    else:
                        nc.vector.tensor_tensor(D[:, :, :], P[:, :, :], X[:, :, :],
                                                Alu.subtract)
                    # z = <x, D> = <h_p, x> - <x, x>   (per chunk, fused mul+sum)
                    if "dotstt" in SKIP:
                        nc.vector.memset(dp[:, :], 0.0)
                    else:
                        for k in range(C):
                            nc.vector.scalar_tensor_tensor(
                                jt[:, :], X[:, k, :], 0.0, D[:, k, :],
                                Alu.bypass, Alu.mult,
                                accum_out=dp[:, k : k + 1])
                    # w = sigmoid(z) = alpha/(alpha+beta)
                    nc.scalar.activation(wh[:, :], dp[:, :], Act.Sigmoid)
                    # h = w*D + x
                    if "stt" in SKIP:
                        nc.scalar.activation(H[:, :, :], X[:, :, :], Act.Copy)
                    else:
                        for k in range(C):
                            nc.vector.scalar_tensor_tensor(
                                H[:, k, :], D[:, k, :], wh[:, k : k + 1], X[:, k, :],
                                Alu.mult, Alu.add)

                if WBSCATTER:
                    nc.gpsimd.dma_scatter_add(
                        state_t[g][:, :], H[:, :, :],
                        wi[:, 8 * off : 8 * (off + C)], n, nreg, DIM,
                        single_packet=SINGLE_PACKET)
                else:
                    dst = state_t[g][PART * off : PART * (off + C)].rearrange(
                        "(c p) e -> p c e", p=PART)
                    nc.sync.dma_start(dst, H[:, :, :])

        stack.close()

    nc.compile()
    return nc


def kernel(tree_embedding, node_connection, node_mask=None):
    import sys
    if "/opt/trn_rl_repo" not in sys.path:
        sys.path.insert(0, "/opt/trn_rl_repo")
    from concourse.bass_utils import run_bass_kernel_spmd

    emb = np.ascontiguousarray(np.asarray(tree_embedding, dtype=np.float32))
    conn = np.asarray(node_connection).astype(np.int32)
    B, N, D = emb.shape
    assert D == DIM and B == N_CORES * STREAMS * TREES_PER_STREAM

    L, Cls, sched = _build_schedule(conn)
    nc = _build_program(L, Cls)

    in_maps = []
    for c in range(N_CORES):
        m = {}
        for g in range(STREAMS):
            trees = sched[c][f"trees{g}"]
            m[f"emb{g}"] = emb[trees].reshape(TREES_PER_STREAM * N, DIM)
            m[f"eidx{g}"] = sched[c][f"eidx{g}"]
            m[f"pidx{g}"] = sched[c][f"pidx{g}"]
            if DYN:
                m[f"cnt{g}"] = sched[c][f"cnt{g}"]
            if WBSCATTER:
                m[f"widx{g}"] = sched[c][f"widx{g}"]
        in_maps.append(m)

    res = run_bass_kernel_spmd(nc, in_maps, list(range(N_CORES)))

    out = np.empty((B, N, DIM), np.float32)
    for c in range(N_CORES):
        for g in range(STREAMS):
            state = res.results[c][f"state{g}"]
            posmat = sched[c][f"posmat{g}"]
            for t, b in enumerate(sched[c][f"trees{g}"]):
                out[b] = state[posmat[t]]
    return out

